# revision 34
# baseline (speedup 1.0000x reference)
"""Multi-head attention (B=4, N=2048, dim=768, H=16, d_k=48) on 8 TRN2 NeuronCores.

Sharding: data-parallel over (batch, query-half): core c handles batch c//2,
queries [1024*(c%2), 1024*(c%2+1)).  K/V are computed per-core for the full
batch element (replicated across the 2 cores sharing a batch), so there are
no collectives.

Layout strategy (all matmuls in bf16, f32 PSUM accumulation):
  - Host pre-packs x^T, and head-pair-padded transposed weights (each head
    padded from 48 to 64 partitions so matmul tile_position stays in {0,64}).
  - Q^T/K^T produced in [head-dim, token] layout; V in [token, head-dim]
    layout augmented with a ones column (so the softmax denominator falls out
    of the P@V matmul for free as an extra output row).
  - Scores are computed transposed: S^T[kt, qt] = K^T.T @ Q^T, so the exp
    eviction (ScalarE, PSUM->SBUF bf16) directly yields P^T tiles which feed
    the A@V matmul as the moving operand; softmax is computed without max
    subtraction (scores are ~N(0,1) here; exp stays in [e-6, e+6]).
  - Per-head normalization multiplies O^T by the replicated reciprocal of the
    denominator row; V-bias and out-bias are folded into a precomputed bias
    row added during the final eviction.
"""

import numpy as np
import ml_dtypes

BF16 = ml_dtypes.bfloat16
DIM = 768
H = 16
DK = 48
B = 4
N = 2048
QH = 1024           # queries per core
NCORES = 8
KT = N // 128       # 16 key tiles
PAIRS = H // 2      # 8 head pairs (one padded 128-row weight tile each)
INV_SQRT_DK = 1.0 / float(np.sqrt(DK))
VPAD = 65          # V columns: 48 data + 16 pad + ones column at 64
SUMROW = 64
ACT_W = 1024       # full exp on ScalarE (DVE PSUM reads contend with PE)
# Schraudolph bf16: bits16 = round(s * SCH_A + SCH_B) reinterpreted as bf16
# approximates exp(s / sqrt(DK)); SCH_B folds the standard -0.0579 correction.
SCH_A = 128.0 * float(np.log2(np.e)) * INV_SQRT_DK
SCH_B = 127.0 * 128.0 - 7.4109

_compiled = None


def _emit(tc, nc):
    import concourse.mybir as mybir
    from concourse.bass import ts

    f32 = mybir.dt.float32
    bf16 = mybir.dt.bfloat16
    fp8 = mybir.dt.float8e4
    i16 = mybir.dt.int16
    Ident = mybir.ActivationFunctionType.Identity
    Exp = mybir.ActivationFunctionType.Exp

    m = nc.m.functions[0]
    # dram handles by name
    dram = {a.memorylocations[0].name: a for a in m.allocations if hasattr(a, "memorylocations")}

    def dp(name):
        return nc.dram_tensor_handles[name].ap()

    xT = dp("xT")
    xqT = dp("xqT")
    wqT = dp("wqT")
    wkT = dp("wkT")
    wvT = dp("wvT")
    woT = dp("woT")
    qb = dp("qb")
    kb = dp("kb")
    vb = dp("vb")
    ob = dp("ob")
    out = dp("out")

    sync = nc.sync

    def _try_skip_ldw(mm_result):
        # second matmul of a same-stationary pair: suppress the redundant
        # LDWEIGHTS if the instruction supports it
        try:
            mm_result.ins.ldweights = False
        except Exception:
            pass

    persist = tc.alloc_tile_pool(name="persist", bufs=1)

    def single(name, shape, dtype):
        return persist.tile(shape, dtype, name=name, tag=name)

    # ---- persistent SBUF tensors ----
    XT = [single(f"XT{j}", [128, N], bf16) for j in range(6)]
    XQT = [single(f"XQT{j}", [128, QH], bf16) for j in range(6)]
    WQ = [single(f"WQ{j}", [128, PAIRS * 128], bf16) for j in range(6)]
    WK = [single(f"WK{j}", [128, PAIRS * 128], bf16) for j in range(6)]
    WV = [single(f"WV{j}", [128, DIM], bf16) for j in range(6)]
    WO = [single(f"WO{j}", [128, DIM], bf16) for j in range(6)]
    QT = [single(f"QT{p}", [128, QH], bf16) for p in range(PAIRS)]
    KTB = [single(f"KTB{p}", [128, N], bf16) for p in range(PAIRS)]
    VT = [single(f"VT{i}", [128, H, VPAD], bf16) for i in range(KT)]
    XA = [single(f"XA{j}", [128, QH], bf16) for j in range(6)]
    qb_sb = single("qb_sb", [128, PAIRS], f32)
    kb_sb = single("kb_sb", [128, PAIRS], f32)
    vb_sb = single("vb_sb", [128, 6], bf16)
    birow = single("birow", [1, DIM], f32)
    birep = single("birep", [128, DIM], f32)
    cneg3 = single("cneg3", [128, 1], f32)

    rs_dram = [nc.dram_tensor(f"rsd{k}", [1, QH], f32).ap() for k in range(2)]
    birow_dram = nc.dram_tensor("birowd", [1, DIM], f32).ap()

    psA = tc.alloc_tile_pool(name="psA", bufs=2, space="PSUM")
    psB = tc.alloc_tile_pool(name="psB", bufs=2, space="PSUM")
    ptp = tc.alloc_tile_pool(name="ptp", bufs=4)
    rsp = tc.alloc_tile_pool(name="rsp", bufs=2)
    xap = tc.alloc_tile_pool(name="xap", bufs=2)
    outp = tc.alloc_tile_pool(name="outp", bufs=1)

    # ---- input DMAs: two HWDGE queues in parallel (SP: Q-path, ACT: K/V-path) ----
    qdma = nc.scalar
    for j in range(6):
        sync.dma_start(out=XQT[j][:], in_=xqT[ts(j, 128), :])
        sync.dma_start(out=WQ[j][:], in_=wqT[ts(j, 128), :, :])
        qdma.dma_start(out=XT[j][:], in_=xT[ts(j, 128), :])
        qdma.dma_start(out=WK[j][:], in_=wkT[ts(j, 128), :, :])
    sync.dma_start(out=qb_sb[:], in_=qb[:, :])
    qdma.dma_start(out=kb_sb[:], in_=kb[:, :])
    for j in range(6):
        qdma.dma_start(out=WV[j][:], in_=wvT[ts(j, 128), :])
    qdma.dma_start(out=vb_sb[:], in_=vb[:, :])
    for j in range(6):
        sync.dma_start(out=WO[j][:], in_=woT[ts(j, 128), :])
    sync.dma_start(out=birep[0:1, :], in_=ob[:, :])

    # zero pad columns, ones in the sums column of V-hat
    for i in range(KT):
        nc.vector.memset(VT[i][:, :, DK:VPAD], 0.0)
        nc.vector.memset(VT[i][:, :, SUMROW:SUMROW + 1], 1.0)

    # ---- phase helpers ----
    # Projections are written as generators yielding after each matmul so the
    # scheduler below can interleave them between attention steps ("fillers"),
    # keeping the PE instruction stream dense (avoids HAM clock oscillation).
    def q_gen(p):
        ps = psB.tile([128, QH], f32, name=f"psQ{p}", tag="PSB")
        for k in range(6):
            for c in range(2):
                nc.tensor.matmul(
                    out=ps[:, ts(c, 512)],
                    lhsT=WQ[k][:, ts(p, 128)],
                    rhs=XQT[k][:, ts(c, 512)],
                    start=(k == 0), stop=(k == 5),
                )
                yield
        nc.scalar.activation(QT[p][:], ps[:], Ident, bias=qb_sb[:, p:p + 1], scale=1.0)
        yield

    def k_gen(p):
        for half in range(2):
            ps = psB.tile([128, QH], f32, name=f"psK{p}_{half}", tag="PSB")
            for k in range(6):
                for c in range(2):
                    nc.tensor.matmul(
                        out=ps[:, ts(c, 512)],
                        lhsT=WK[k][:, ts(p, 128)],
                        rhs=XT[k][:, ts(2 * half + c, 512)],
                        start=(k == 0), stop=(k == 5),
                    )
                    yield
            nc.scalar.activation(
                KTB[p][:, ts(half, QH)], ps[:], Ident, bias=kb_sb[:, p:p + 1], scale=1.0
            )
            yield

    def v_gen(i):
        ps = psB.tile([128, QH], f32, name=f"psV{i}", tag="PSB")
        for k in range(6):
            for cc, (base, h0) in enumerate([(0, 0), (512, 8)]):
                nc.tensor.matmul(
                    out=ps[:, base:base + 384],
                    lhsT=XT[k][:, ts(i, 128)],
                    rhs=WV[k][:, h0 * DK:h0 * DK + 384],
                    start=(k == 0), stop=(k == 5),
                )
                yield
        for cc, (base, h0) in enumerate([(0, 0), (512, 8)]):
            nc.vector.tensor_copy(
                VT[i][:, h0:h0 + 8, 0:DK],
                ps[:, base:base + 384].rearrange("p (h d) -> p h d", h=8),
            )
        yield

    def bias_gen():
        ps = psB.tile([1, DIM], f32, name="psBias", tag="PSB")
        for c, (base, w) in enumerate([(0, 512), (512, 256)]):
            for k in range(6):
                nc.tensor.matmul(
                    out=ps[:, base:base + w],
                    lhsT=vb_sb[:, k:k + 1],
                    rhs=WO[k][:, base:base + w],
                    start=(k == 0), stop=(k == 5),
                )
                yield
        nc.vector.tensor_add(birow[:], ps[:], birep[0:1, :])
        sync.dma_start(out=birow_dram[:], in_=birow[:])
        sync.dma_start(out=birep[:], in_=birow_dram[:].partition_broadcast(128))
        yield

    psO_of = {}
    pt_of = {}

    def scores(h, i):
        p = h // 2
        off = 64 * (h % 2)
        psS = psA.tile([128, QH], f32, name=f"psS{h}_{i}", tag="PSA")
        for c in range(2):
            nc.tensor.matmul(
                out=psS[:, ts(c, 512)],
                lhsT=KTB[p][off:off + DK, ts(i, 128)],
                rhs=QT[p][off:off + DK, ts(c, 512)],
                start=True, stop=True,
            )
        pt = ptp.tile([128, QH], bf16, name=f"pt{h}_{i}", tag="PT")
        pt_of[(h, i)] = pt
        nc.scalar.activation(pt[:], psS[:], Exp, scale=INV_SQRT_DK)

    def av(h, i):
        if i == 0:
            psO_of[h] = psB.tile([VPAD, QH], f32, name=f"psO{h}", tag="PSB")
        psO = psO_of[h]
        pt = pt_of.pop((h, i))
        for c in range(2):
            nc.tensor.matmul(
                out=psO[:, ts(c, 512)],
                lhsT=VT[i][:, h, :],
                rhs=pt[:, ts(c, 512)],
                start=(i == 0), stop=(i == KT - 1),
            )

    def norm(h):
        # normalization: replicate the sums row across 48 partitions via a
        # DRAM bounce (SBUF DMA sources cannot have partition step 0), then
        # reciprocal at partition base 0 (custom-DVE op requires base 0)
        psO = psO_of.pop(h)
        rs = rsp.tile([VPAD, QH], f32, name=f"rs{h}", tag="RS")
        nc.vector.tensor_copy(rs[SUMROW:SUMROW + 1, :], psO[SUMROW:SUMROW + 1, :])
        rsd = rs_dram[h % 2]
        sync.dma_start(out=rsd[:], in_=rs[SUMROW:SUMROW + 1, :])
        sync.dma_start(out=rs[0:DK, :], in_=rsd[:].partition_broadcast(DK))
        nc.vector.reciprocal_approx_fast(out=rs[0:DK, :], in_=rs[0:DK, :])
        xa = xap.tile([DK, QH], bf16, name=f"xa{h}", tag="XAH")
        nc.vector.tensor_mul(xa[:], psO[0:DK, :], rs[0:DK, :])
        # scatter head rows into the f-major X_att^T tiles (partition shift via DMA)
        r = DK * h
        f0, r0 = r // 128, r % 128
        n1 = min(128 - r0, DK)
        sync.dma_start(out=XA[f0][r0:r0 + n1, :], in_=xa[0:n1, :])
        if n1 < DK:
            sync.dma_start(out=XA[f0 + 1][0:DK - n1, :], in_=xa[n1:DK, :])

    def out_proj_04(t):
        # f-chunks 0..4 only touch heads <= 13, so these run while the last
        # heads' normalization chains drain; alternate PSUM pools (both are
        # free by now) for a deeper tail pipeline
        # odd tiles take psA (free right after the last scores); even tiles
        # take psB (t=0 early in head 15; others as psO slots free)
        use_a = (t % 2 == 1)
        pool = psA if use_a else psB
        ps = pool.tile([128, QH], f32, name=f"psY{t}", tag="PSA" if use_a else "PSB")
        for k in range(5):
            for c, (base, w) in enumerate([(0, 512), (512, 256)]):
                nc.tensor.matmul(
                    out=ps[:, base:base + w],
                    lhsT=XA[k][:, ts(t, 128)],
                    rhs=WO[k][:, base:base + w],
                    start=(k == 0), stop=False,
                )
        return ps

    def out_proj_5(t, ps):
        for c, (base, w) in enumerate([(0, 512), (512, 256)]):
            nc.tensor.matmul(
                out=ps[:, base:base + w],
                lhsT=XA[5][:, ts(t, 128)],
                rhs=WO[5][:, base:base + w],
                start=False, stop=True,
            )
        o = outp.tile([128, DIM], f32, name=f"o{t}", tag="OUT")
        nc.vector.tensor_add(o[:], ps[:, 0:DIM], birep[:])
        sync.dma_start(out=out[ts(t, 128), :], in_=o[:])

    # ---- schedule: lag-2 scores/AV software pipeline with proj fillers ----
    from collections import deque

    fillers = deque()

    def pump(n):
        done = 0
        while fillers and done < n:
            try:
                next(fillers[0])
                done += 1
            except StopIteration:
                fillers.popleft()

    for g in (q_gen(0), k_gen(0), v_gen(0)):
        for _ in g:
            pass

    for i in range(1, KT):
        fillers.append(v_gen(i))

    av_q = deque()
    ps_early = {}
    for h in range(H):
        # just-in-time fillers: pair p's Q/K land during heads 2p-2 and 2p-1
        if h == 1:
            fillers.append(q_gen(1))
            fillers.append(k_gen(1))
        elif h >= 2 and h % 2 == 0 and h // 2 + 1 < PAIRS:
            fillers.append(q_gen(h // 2 + 1))
            fillers.append(k_gen(h // 2 + 1))
        if h == 14:
            fillers.append(bias_gen())
        budget = 13 if h == 0 else (3 if h == 1 else 2)
        for i in range(KT):
            scores(h, i)
            pump(budget)
            av_q.append((h, i))
            if len(av_q) > 2:
                hh, ii = av_q.popleft()
                av(hh, ii)
                if ii == KT - 1:
                    norm(hh)
                    if hh == H - 2:
                        # head 14's psO slot is free; its Y04 fills the
                        # filler-starved head-15 stream
                        ps_early[0] = out_proj_04(0)
    while av_q:
        hh, ii = av_q.popleft()
        av(hh, ii)
        if ii == KT - 1:
            norm(hh)
    pump(10 ** 9)
    ps_prev = ps_early[0]
    for t in range(1, QH // 128):
        ps_t = out_proj_04(t)
        out_proj_5(t - 1, ps_prev)
        ps_prev = ps_t
    out_proj_5(QH // 128 - 1, ps_prev)

    for pool in (outp, xap, rsp, ptp, psB, psA, persist):
        pool.release()


def _build():
    import concourse.mybir as mybir
    import concourse.tile as tile
    from concourse import bacc

    f32 = mybir.dt.float32
    bf16 = mybir.dt.bfloat16

    nc = bacc.Bacc("TRN2", target_bir_lowering=False, debug=False, num_devices=NCORES)
    nc.dram_tensor_handles = {}

    def decl(name, shape, dtype, is_out=False):
        h = nc.declare_dram_parameter(name, list(shape), dtype, isOutput=is_out)
        nc.dram_tensor_handles[name] = h
        return h

    decl("xT", [DIM, N], bf16)
    decl("xqT", [DIM, QH], bf16)
    decl("wqT", [DIM, PAIRS, 128], bf16)
    decl("wkT", [DIM, PAIRS, 128], bf16)
    decl("wvT", [DIM, DIM], bf16)
    decl("woT", [DIM, DIM], bf16)
    decl("qb", [128, PAIRS], f32)
    decl("kb", [128, PAIRS], f32)
    decl("vb", [128, 6], bf16)
    decl("ob", [1, DIM], f32)
    decl("out", [QH, DIM], f32, is_out=True)

    with tile.TileContext(nc) as tc:
        _emit(tc, nc)
    nc.compile()
    return nc


def _host_prep(x, qkv_w, qkv_b, out_w, out_b):
    x = np.asarray(x, np.float32)
    qkv_w = np.asarray(qkv_w, np.float32)
    qkv_b = np.asarray(qkv_b, np.float32)
    out_w = np.asarray(out_w, np.float32)
    out_b = np.asarray(out_b, np.float32)

    wq, wk = qkv_w[0:DIM], qkv_w[DIM:2 * DIM]
    wv = qkv_w[2 * DIM:3 * DIM]

    def pack_pairs(w):  # w: [768(out), 768(in)] -> [768(in), 8, 128] padded
        wT = w.T
        out_arr = np.zeros((DIM, PAIRS, 128), np.float32)
        for j in range(PAIRS):
            out_arr[:, j, 0:DK] = wT[:, 96 * j:96 * j + DK]
            out_arr[:, j, 64:64 + DK] = wT[:, 96 * j + DK:96 * j + 96]
        return out_arr.astype(BF16)

    def pack_bias(bvec):  # [768] -> [128, 8] padded
        out_arr = np.zeros((128, PAIRS), np.float32)
        for j in range(PAIRS):
            out_arr[0:DK, j] = bvec[96 * j:96 * j + DK]
            out_arr[64:64 + DK, j] = bvec[96 * j + DK:96 * j + 96]
        return out_arr

    common = {
        "wqT": pack_pairs(wq),
        "wkT": pack_pairs(wk),
        "wvT": np.ascontiguousarray(wv.T).astype(BF16),
        "woT": np.ascontiguousarray(out_w.T).astype(BF16),
        "qb": pack_bias(qkv_b[0:DIM]),
        "kb": pack_bias(qkv_b[DIM:2 * DIM]),
        "vb": np.ascontiguousarray(qkv_b[2 * DIM:].reshape(6, 128).T).astype(BF16),
        "ob": out_b.reshape(1, DIM).astype(np.float32),
    }
    xT_all = np.ascontiguousarray(x.transpose(0, 2, 1)).astype(BF16)  # [B, 768, N]
    in_maps = []
    for c in range(NCORES):
        b, qh = c // 2, c % 2
        mcore = dict(common)
        mcore["xT"] = xT_all[b]
        mcore["xqT"] = np.ascontiguousarray(xT_all[b][:, qh * QH:(qh + 1) * QH])
        in_maps.append(mcore)
    return in_maps


def _run(in_maps, trace=False):
    global _compiled
    from concourse.bass_utils import run_bass_kernel_spmd

    if _compiled is None:
        _compiled = _build()
    return run_bass_kernel_spmd(_compiled, in_maps, list(range(NCORES)), trace=trace)


def kernel(x, qkv_w, qkv_b, out_w, out_b):
    in_maps = _host_prep(x, qkv_w, qkv_b, out_w, out_b)
    res = _run(in_maps, trace=False)
    out = np.empty((B, N, DIM), np.float32)
    for c in range(NCORES):
        b, qh = c // 2, c % 2
        out[b, qh * QH:(qh + 1) * QH] = res.results[c]["out"]
    return out


# revision 35
# speedup vs baseline: 1.0273x; 1.0273x over previous
"""Multi-head attention (B=4, N=2048, dim=768, H=16, d_k=48) on 8 TRN2 NeuronCores.

Sharding: data-parallel over (batch, query-half): core c handles batch c//2,
queries [1024*(c%2), 1024*(c%2+1)).  K/V are computed per-core for the full
batch element (replicated across the 2 cores sharing a batch), so there are
no collectives.

Layout strategy (all matmuls in bf16, f32 PSUM accumulation):
  - Host pre-packs x^T, and head-pair-padded transposed weights (each head
    padded from 48 to 64 partitions so matmul tile_position stays in {0,64}).
  - Q^T/K^T produced in [head-dim, token] layout; V in [token, head-dim]
    layout augmented with a ones column (so the softmax denominator falls out
    of the P@V matmul for free as an extra output row).
  - Scores are computed transposed: S^T[kt, qt] = K^T.T @ Q^T, so the exp
    eviction (ScalarE, PSUM->SBUF bf16) directly yields P^T tiles which feed
    the A@V matmul as the moving operand; softmax is computed without max
    subtraction (scores are ~N(0,1) here; exp stays in [e-6, e+6]).
  - Per-head normalization multiplies O^T by the replicated reciprocal of the
    denominator row; V-bias and out-bias are folded into a precomputed bias
    row added during the final eviction.
"""

import numpy as np
import ml_dtypes

BF16 = ml_dtypes.bfloat16
DIM = 768
H = 16
DK = 48
B = 4
N = 2048
QH = 1024           # queries per core
NCORES = 8
KT = N // 128       # 16 key tiles
PAIRS = H // 2      # 8 head pairs (one padded 128-row weight tile each)
INV_SQRT_DK = 1.0 / float(np.sqrt(DK))
VPAD = 65          # V columns: 48 data + 16 pad + ones column at 64
SUMROW = 64
ACT_W = 1024       # full exp on ScalarE (DVE PSUM reads contend with PE)
# Schraudolph bf16: bits16 = round(s * SCH_A + SCH_B) reinterpreted as bf16
# approximates exp(s / sqrt(DK)); SCH_B folds the standard -0.0579 correction.
SCH_A = 128.0 * float(np.log2(np.e)) * INV_SQRT_DK
SCH_B = 127.0 * 128.0 - 7.4109

_compiled = None


def _emit(tc, nc):
    import concourse.mybir as mybir
    from concourse.bass import ts

    f32 = mybir.dt.float32
    bf16 = mybir.dt.bfloat16
    fp8 = mybir.dt.float8e4
    i16 = mybir.dt.int16
    Ident = mybir.ActivationFunctionType.Identity
    Exp = mybir.ActivationFunctionType.Exp

    m = nc.m.functions[0]
    # dram handles by name
    dram = {a.memorylocations[0].name: a for a in m.allocations if hasattr(a, "memorylocations")}

    def dp(name):
        return nc.dram_tensor_handles[name].ap()

    xT = dp("xT")
    xqT = dp("xqT")
    wqT = dp("wqT")
    wkT = dp("wkT")
    wvT = dp("wvT")
    woT = dp("woT")
    qb = dp("qb")
    kb = dp("kb")
    vb = dp("vb")
    ob = dp("ob")
    out = dp("out")

    sync = nc.sync

    def _try_skip_ldw(mm_result):
        # second matmul of a same-stationary pair: suppress the redundant
        # LDWEIGHTS if the instruction supports it
        try:
            mm_result.ins.ldweights = False
        except Exception:
            pass

    persist = tc.alloc_tile_pool(name="persist", bufs=1)

    def single(name, shape, dtype):
        return persist.tile(shape, dtype, name=name, tag=name)

    # ---- persistent SBUF tensors ----
    XT = [single(f"XT{j}", [128, N], bf16) for j in range(6)]
    XQT = [single(f"XQT{j}", [128, QH], bf16) for j in range(6)]
    WQ = [single(f"WQ{j}", [128, PAIRS * 128], bf16) for j in range(6)]
    WK = [single(f"WK{j}", [128, PAIRS * 128], bf16) for j in range(6)]
    WV = [single(f"WV{j}", [128, DIM], bf16) for j in range(6)]
    WO = [single(f"WO{j}", [128, DIM], bf16) for j in range(6)]
    QT = [single(f"QT{p}", [128, QH], bf16) for p in range(PAIRS)]
    KTB = [single(f"KTB{p}", [128, N], bf16) for p in range(PAIRS)]
    VT = [single(f"VT{i}", [128, H, VPAD], bf16) for i in range(KT)]
    XA = [single(f"XA{j}", [128, QH], bf16) for j in range(6)]
    qb_sb = single("qb_sb", [128, PAIRS], f32)
    kb_sb = single("kb_sb", [128, PAIRS], f32)
    vb_sb = single("vb_sb", [128, 6], bf16)
    birow = single("birow", [1, DIM], f32)
    birep = single("birep", [128, DIM], f32)
    cneg3 = single("cneg3", [128, 1], f32)

    rs_dram = [nc.dram_tensor(f"rsd{k}", [1, QH], f32).ap() for k in range(2)]
    birow_dram = nc.dram_tensor("birowd", [1, DIM], f32).ap()

    psA = tc.alloc_tile_pool(name="psA", bufs=2, space="PSUM")
    psB = tc.alloc_tile_pool(name="psB", bufs=2, space="PSUM")
    ptp = tc.alloc_tile_pool(name="ptp", bufs=4)
    rsp = tc.alloc_tile_pool(name="rsp", bufs=2)
    xap = tc.alloc_tile_pool(name="xap", bufs=2)
    outp = tc.alloc_tile_pool(name="outp", bufs=1)

    # ---- input DMAs: two HWDGE queues in parallel (SP: Q-path, ACT: K/V-path) ----
    qdma = nc.scalar
    for j in range(6):
        sync.dma_start(out=XQT[j][:], in_=xqT[ts(j, 128), :])
        qdma.dma_start(out=XT[j][:], in_=xT[ts(j, 128), :])
    for j in range(6):
        sync.dma_start(out=WQ[j][:], in_=wqT[ts(j, 128), :, :])
        qdma.dma_start(out=WK[j][:], in_=wkT[ts(j, 128), :, :])
    sync.dma_start(out=qb_sb[:], in_=qb[:, :])
    qdma.dma_start(out=kb_sb[:], in_=kb[:, :])
    for j in range(6):
        qdma.dma_start(out=WV[j][:], in_=wvT[ts(j, 128), :])
    qdma.dma_start(out=vb_sb[:], in_=vb[:, :])
    for j in range(6):
        sync.dma_start(out=WO[j][:], in_=woT[ts(j, 128), :])
    sync.dma_start(out=birep[0:1, :], in_=ob[:, :])

    # zero pad columns, ones in the sums column of V-hat
    for i in range(KT):
        nc.vector.memset(VT[i][:, :, DK:VPAD], 0.0)
        nc.vector.memset(VT[i][:, :, SUMROW:SUMROW + 1], 1.0)

    # ---- phase helpers ----
    # Projections are written as generators yielding after each matmul so the
    # scheduler below can interleave them between attention steps ("fillers"),
    # keeping the PE instruction stream dense (avoids HAM clock oscillation).
    def q_gen(p):
        ps = psB.tile([128, QH], f32, name=f"psQ{p}", tag="PSB")
        for k in range(6):
            for c in range(2):
                nc.tensor.matmul(
                    out=ps[:, ts(c, 512)],
                    lhsT=WQ[k][:, ts(p, 128)],
                    rhs=XQT[k][:, ts(c, 512)],
                    start=(k == 0), stop=(k == 5),
                )
                yield
        nc.scalar.activation(QT[p][:], ps[:], Ident, bias=qb_sb[:, p:p + 1], scale=1.0)
        yield

    def k_gen(p):
        for half in range(2):
            ps = psB.tile([128, QH], f32, name=f"psK{p}_{half}", tag="PSB")
            for k in range(6):
                for c in range(2):
                    nc.tensor.matmul(
                        out=ps[:, ts(c, 512)],
                        lhsT=WK[k][:, ts(p, 128)],
                        rhs=XT[k][:, ts(2 * half + c, 512)],
                        start=(k == 0), stop=(k == 5),
                    )
                    yield
            nc.scalar.activation(
                KTB[p][:, ts(half, QH)], ps[:], Ident, bias=kb_sb[:, p:p + 1], scale=1.0
            )
            yield

    def v_gen(i):
        ps = psB.tile([128, QH], f32, name=f"psV{i}", tag="PSB")
        for k in range(6):
            for cc, (base, h0) in enumerate([(0, 0), (512, 8)]):
                nc.tensor.matmul(
                    out=ps[:, base:base + 384],
                    lhsT=XT[k][:, ts(i, 128)],
                    rhs=WV[k][:, h0 * DK:h0 * DK + 384],
                    start=(k == 0), stop=(k == 5),
                )
                yield
        for cc, (base, h0) in enumerate([(0, 0), (512, 8)]):
            nc.vector.tensor_copy(
                VT[i][:, h0:h0 + 8, 0:DK],
                ps[:, base:base + 384].rearrange("p (h d) -> p h d", h=8),
            )
        yield

    def bias_gen():
        ps = psB.tile([1, DIM], f32, name="psBias", tag="PSB")
        for c, (base, w) in enumerate([(0, 512), (512, 256)]):
            for k in range(6):
                nc.tensor.matmul(
                    out=ps[:, base:base + w],
                    lhsT=vb_sb[:, k:k + 1],
                    rhs=WO[k][:, base:base + w],
                    start=(k == 0), stop=(k == 5),
                )
                yield
        nc.vector.tensor_add(birow[:], ps[:], birep[0:1, :])
        sync.dma_start(out=birow_dram[:], in_=birow[:])
        sync.dma_start(out=birep[:], in_=birow_dram[:].partition_broadcast(128))
        yield

    psO_of = {}
    pt_of = {}

    def scores(h, i):
        p = h // 2
        off = 64 * (h % 2)
        psS = psA.tile([128, QH], f32, name=f"psS{h}_{i}", tag="PSA")
        for c in range(2):
            nc.tensor.matmul(
                out=psS[:, ts(c, 512)],
                lhsT=KTB[p][off:off + DK, ts(i, 128)],
                rhs=QT[p][off:off + DK, ts(c, 512)],
                start=True, stop=True,
            )
        pt = ptp.tile([128, QH], bf16, name=f"pt{h}_{i}", tag="PT")
        pt_of[(h, i)] = pt
        nc.scalar.activation(pt[:], psS[:], Exp, scale=INV_SQRT_DK)

    def av(h, i):
        if i == 0:
            psO_of[h] = psB.tile([VPAD, QH], f32, name=f"psO{h}", tag="PSB")
        psO = psO_of[h]
        pt = pt_of.pop((h, i))
        for c in range(2):
            nc.tensor.matmul(
                out=psO[:, ts(c, 512)],
                lhsT=VT[i][:, h, :],
                rhs=pt[:, ts(c, 512)],
                start=(i == 0), stop=(i == KT - 1),
            )

    def norm(h):
        # normalization: replicate the sums row across 48 partitions via a
        # DRAM bounce (SBUF DMA sources cannot have partition step 0), then
        # reciprocal at partition base 0 (custom-DVE op requires base 0)
        psO = psO_of.pop(h)
        rs = rsp.tile([VPAD, QH], f32, name=f"rs{h}", tag="RS")
        nc.vector.tensor_copy(rs[SUMROW:SUMROW + 1, :], psO[SUMROW:SUMROW + 1, :])
        rsd = rs_dram[h % 2]
        sync.dma_start(out=rsd[:], in_=rs[SUMROW:SUMROW + 1, :])
        sync.dma_start(out=rs[0:DK, :], in_=rsd[:].partition_broadcast(DK))
        nc.vector.reciprocal_approx_fast(out=rs[0:DK, :], in_=rs[0:DK, :])
        xa = xap.tile([DK, QH], bf16, name=f"xa{h}", tag="XAH")
        nc.vector.tensor_mul(xa[:], psO[0:DK, :], rs[0:DK, :])
        # scatter head rows into the f-major X_att^T tiles (partition shift via DMA)
        r = DK * h
        f0, r0 = r // 128, r % 128
        n1 = min(128 - r0, DK)
        sync.dma_start(out=XA[f0][r0:r0 + n1, :], in_=xa[0:n1, :])
        if n1 < DK:
            sync.dma_start(out=XA[f0 + 1][0:DK - n1, :], in_=xa[n1:DK, :])

    def out_proj_04(t):
        # f-chunks 0..4 only touch heads <= 13, so these run while the last
        # heads' normalization chains drain; alternate PSUM pools (both are
        # free by now) for a deeper tail pipeline
        ps = psA.tile([128, QH], f32, name=f"psY{t}", tag="PSA")
        for k in range(5):
            for c, (base, w) in enumerate([(0, 512), (512, 256)]):
                nc.tensor.matmul(
                    out=ps[:, base:base + w],
                    lhsT=XA[k][:, ts(t, 128)],
                    rhs=WO[k][:, base:base + w],
                    start=(k == 0), stop=False,
                )
        return ps

    def out_proj_5(t, ps):
        for c, (base, w) in enumerate([(0, 512), (512, 256)]):
            nc.tensor.matmul(
                out=ps[:, base:base + w],
                lhsT=XA[5][:, ts(t, 128)],
                rhs=WO[5][:, base:base + w],
                start=False, stop=True,
            )
        o = outp.tile([128, DIM], f32, name=f"o{t}", tag="OUT")
        nc.vector.tensor_add(o[:], ps[:, 0:DIM], birep[:])
        sync.dma_start(out=out[ts(t, 128), :], in_=o[:])

    # ---- schedule: lag-2 scores/AV software pipeline with proj fillers ----
    from collections import deque

    fillers = deque()

    def pump(n):
        done = 0
        while fillers and done < n:
            try:
                next(fillers[0])
                done += 1
            except StopIteration:
                fillers.popleft()

    for g in (q_gen(0), k_gen(0), v_gen(0)):
        for _ in g:
            pass

    for i in range(1, KT):
        fillers.append(v_gen(i))

    av_q = deque()
    for h in range(H):
        # just-in-time fillers: pair p's Q/K land during heads 2p-2 and 2p-1
        if h == 1:
            fillers.append(q_gen(1))
            fillers.append(k_gen(1))
        elif h >= 2 and h % 2 == 0 and h // 2 + 1 < PAIRS:
            fillers.append(q_gen(h // 2 + 1))
            fillers.append(k_gen(h // 2 + 1))
        if h == 14:
            fillers.append(bias_gen())
        budget = 13 if h == 0 else (3 if h == 1 else 2)
        for i in range(KT):
            scores(h, i)
            pump(budget)
            av_q.append((h, i))
            if len(av_q) > 2:
                hh, ii = av_q.popleft()
                av(hh, ii)
                if ii == KT - 1:
                    norm(hh)
    while av_q:
        hh, ii = av_q.popleft()
        av(hh, ii)
        if ii == KT - 1:
            norm(hh)
    pump(10 ** 9)
    ps_prev = None
    for t in range(QH // 128):
        ps_t = out_proj_04(t)
        if ps_prev is not None:
            out_proj_5(t - 1, ps_prev)
        ps_prev = ps_t
    out_proj_5(QH // 128 - 1, ps_prev)

    for pool in (outp, xap, rsp, ptp, psB, psA, persist):
        pool.release()


def _build():
    import concourse.mybir as mybir
    import concourse.tile as tile
    from concourse import bacc

    f32 = mybir.dt.float32
    bf16 = mybir.dt.bfloat16

    nc = bacc.Bacc("TRN2", target_bir_lowering=False, debug=False, num_devices=NCORES)
    nc.dram_tensor_handles = {}

    def decl(name, shape, dtype, is_out=False):
        h = nc.declare_dram_parameter(name, list(shape), dtype, isOutput=is_out)
        nc.dram_tensor_handles[name] = h
        return h

    decl("xT", [DIM, N], bf16)
    decl("xqT", [DIM, QH], bf16)
    decl("wqT", [DIM, PAIRS, 128], bf16)
    decl("wkT", [DIM, PAIRS, 128], bf16)
    decl("wvT", [DIM, DIM], bf16)
    decl("woT", [DIM, DIM], bf16)
    decl("qb", [128, PAIRS], f32)
    decl("kb", [128, PAIRS], f32)
    decl("vb", [128, 6], bf16)
    decl("ob", [1, DIM], f32)
    decl("out", [QH, DIM], f32, is_out=True)

    with tile.TileContext(nc) as tc:
        _emit(tc, nc)
    nc.compile()
    return nc


def _host_prep(x, qkv_w, qkv_b, out_w, out_b):
    x = np.asarray(x, np.float32)
    qkv_w = np.asarray(qkv_w, np.float32)
    qkv_b = np.asarray(qkv_b, np.float32)
    out_w = np.asarray(out_w, np.float32)
    out_b = np.asarray(out_b, np.float32)

    wq, wk = qkv_w[0:DIM], qkv_w[DIM:2 * DIM]
    wv = qkv_w[2 * DIM:3 * DIM]

    def pack_pairs(w):  # w: [768(out), 768(in)] -> [768(in), 8, 128] padded
        wT = w.T
        out_arr = np.zeros((DIM, PAIRS, 128), np.float32)
        for j in range(PAIRS):
            out_arr[:, j, 0:DK] = wT[:, 96 * j:96 * j + DK]
            out_arr[:, j, 64:64 + DK] = wT[:, 96 * j + DK:96 * j + 96]
        return out_arr.astype(BF16)

    def pack_bias(bvec):  # [768] -> [128, 8] padded
        out_arr = np.zeros((128, PAIRS), np.float32)
        for j in range(PAIRS):
            out_arr[0:DK, j] = bvec[96 * j:96 * j + DK]
            out_arr[64:64 + DK, j] = bvec[96 * j + DK:96 * j + 96]
        return out_arr

    common = {
        "wqT": pack_pairs(wq),
        "wkT": pack_pairs(wk),
        "wvT": np.ascontiguousarray(wv.T).astype(BF16),
        "woT": np.ascontiguousarray(out_w.T).astype(BF16),
        "qb": pack_bias(qkv_b[0:DIM]),
        "kb": pack_bias(qkv_b[DIM:2 * DIM]),
        "vb": np.ascontiguousarray(qkv_b[2 * DIM:].reshape(6, 128).T).astype(BF16),
        "ob": out_b.reshape(1, DIM).astype(np.float32),
    }
    xT_all = np.ascontiguousarray(x.transpose(0, 2, 1)).astype(BF16)  # [B, 768, N]
    in_maps = []
    for c in range(NCORES):
        b, qh = c // 2, c % 2
        mcore = dict(common)
        mcore["xT"] = xT_all[b]
        mcore["xqT"] = np.ascontiguousarray(xT_all[b][:, qh * QH:(qh + 1) * QH])
        in_maps.append(mcore)
    return in_maps


def _run(in_maps, trace=False):
    global _compiled
    from concourse.bass_utils import run_bass_kernel_spmd

    if _compiled is None:
        _compiled = _build()
    return run_bass_kernel_spmd(_compiled, in_maps, list(range(NCORES)), trace=trace)


def kernel(x, qkv_w, qkv_b, out_w, out_b):
    in_maps = _host_prep(x, qkv_w, qkv_b, out_w, out_b)
    res = _run(in_maps, trace=False)
    out = np.empty((B, N, DIM), np.float32)
    for c in range(NCORES):
        b, qh = c // 2, c % 2
        out[b, qh * QH:(qh + 1) * QH] = res.results[c]["out"]
    return out
